# revision 52
# baseline (speedup 1.0000x reference)
"""Trainium2 Bass kernel for the GCM (global context module) problem.

Computation per batch sample b (x_b = x[b] viewed as [C=512, HW=9216]):
    x1 = w1 @ x_b                      [128, HW]
    v  = softmax_all(x1 @ x2^T)        [128, 256]  (softmax over all 32768)
    n  = relu(v + w3 @ v)              [128, 256]
    z  = w4 @ n^T                      [256, 128]
    W  = w5 @ z                        [512, 128]  (collapses y/conv5: w5@(z@x1) == (w5@z)@x1)
    out = x_b + W @ x1                 [512, HW]

Sharding: data-parallel over batch, one sample per NeuronCore (8 cores).

v4 strategy: bf16 I/O + Gram-trick phase 1 with an fp8 x^T operand.
  - The x2 GEMM (2/3 of phase-1 FLOPs) is eliminated:
        v = x1 @ x2^T = (x1 @ x^T) @ w2^T = A @ w2^T
    A is accumulated on the PE from fp8 operands (x1T subtiles produced by
    on-chip PE transposes of the k-major x1, x^T uploaded host-transposed
    in fp8).  Phase-1 PE work drops from 1.81G+0.30G MACs (x1,x2 hw-major
    + v) to 0.60G (x1 k-major, long streams) + 0.15G (transposes) + 0.60G
    (A) + small.
  - Numerics: the softmax is a near-one-hot argmax (top-2 logit gaps
    6.7..102 across the batch) and |x_res| <= 0.04 vs |x| <= 5.1, so fp8
    noise in the logits is harmless; measured end-to-end rel err ~5e-3 vs
    the 2e-2 gate, dominated by the bf16 x/out passthrough rounding.
  - x uploaded bf16 host-repacked (one [128, 36864] SBUF tile, block-major
    with finer-grained block 0, 8 line-rate DMAs); x^T uploaded fp8
    host-transposed ([128, 36864], 6 DMAs) interleaved block-by-block.
  - out written bf16, staged [128,1536], all out-DMAs on the sync HWDGE
    queue; residual add balanced across PE / DVE / ACT.
  - PE warmup against a memset tile (no DMA dependency); dummy matmuls
    bridge the serial softmax window to keep the HAM clock-gate released.
"""

import numpy as np
import ml_dtypes

import concourse.bass as bass
import concourse.tile as tile
from concourse import bacc, mybir, bass_isa
from concourse.bass_utils import run_bass_kernel_spmd

F32 = mybir.dt.float32
BF16 = mybir.dt.bfloat16
FP8 = mybir.dt.float8e4
AX = mybir.AxisListType
AL = mybir.AluOpType
AF = mybir.ActivationFunctionType

N_CORES = 8
C = 512
H = W_IMG = 96
HW = H * W_IMG          # 9216
CK = C // 128           # 4 chunks of channels
NBLK = 6                # x blocks along hw
BLK = HW // NBLK        # 1536
NSUB = HW // 128        # 72 subtiles
NT = HW // 512          # 18 hw tiles of 512
C4 = C // 4             # 128
C2 = C // 2             # 256
KM = C4 + C2            # 384 = concat(w1T, w2T) free size
XCOLS = CK * HW         # free size of the packed x tile (36864)


def _xcol(c, goff):
    """Column in the packed x tile for channel-chunk c, global hw offset."""
    b, off = divmod(goff, BLK)
    if b == 0:
        sub, o = divmod(off, 512)
        return sub * (CK * 512) + c * 512 + o
    return b * (CK * BLK) + c * BLK + off


def _emit(ctx, tc, aps, use_bias):
    nc = tc.nc
    x_d = aps["x"]
    xt8_d = aps["xt8"]
    w12t_d = aps["w12t"]
    w3t_d = aps["w3t"]
    w4t_d = aps["w4t"]
    w5t_d = aps["w5t"]
    out_d = aps["out"]

    consts = ctx.enter_context(tc.tile_pool(name="consts", bufs=1))

    # Warmup operand: memset, so the first PE matmuls have no DMA dependency.
    warm0 = consts.tile([128, 128], BF16, tag="warm0")
    nc.vector.memset(warm0[:], 0)

    # ---- input stream on sync: w1T (tiny, gates phase-1 start), then x
    # block 0 in 3 sub-blocks, then x / x^T blocks interleaved.  The
    # scalar ring only carries small late-needed constants (bulk
    # transfers on it are served far slower than on sync).
    identb = consts.tile([128, 128], BF16, tag="identb")
    nc.scalar.dma_start(out=identb[:], in_=aps["identb"][:, :])
    identf = consts.tile([128, 128], F32, tag="identf")
    nc.scalar.dma_start(out=identf[:], in_=aps["identf"][:, :])
    w1c = []
    for c in range(CK):
        t = consts.tile([128, C4], BF16, tag=f"w1_{c}")
        nc.sync.dma_start(out=t[:], in_=w12t_d[c * 128 : (c + 1) * 128, 0:C4])
        w1c.append(t)

    # x resident: one packed [128, 36864] bf16 tile; x^T fp8 likewise.
    # Interleave xT blocks behind the matching x blocks (A-matmuls for
    # block b trail the x1-matmuls by two pipeline stages).
    xpool = ctx.enter_context(tc.tile_pool(name="x", bufs=1))
    xall = xpool.tile([128, XCOLS], BF16, tag="xall", name="xall")
    xt8 = xpool.tile([128, XCOLS], FP8, tag="xt8", name="xt8")
    x1sb = xpool.tile([128, HW], BF16, tag="x1sb", name="x1sb")
    WSUB = CK * 512
    for sub in range(3):
        nc.sync.dma_start(
            out=xall[:, sub * WSUB : (sub + 1) * WSUB],
            in_=x_d[:, sub * WSUB : (sub + 1) * WSUB],
        )
    WBLK = CK * BLK

    def _xt8_block(b):
        nc.sync.dma_start(
            out=xt8[:, b * WBLK : (b + 1) * WBLK],
            in_=xt8_d[:, b * WBLK : (b + 1) * WBLK],
        )

    _xt8_block(0)
    for b in range(1, NBLK):
        nc.sync.dma_start(
            out=xall[:, b * WBLK : (b + 1) * WBLK],
            in_=x_d[:, b * WBLK : (b + 1) * WBLK],
        )
        _xt8_block(b)

    def xs(c, goff, width):
        col = _xcol(c, goff)
        return xall[:, col : col + width]
    w2c = []
    for c in range(CK):
        t = consts.tile([128, C2], BF16, tag=f"w2_{c}")
        nc.scalar.dma_start(out=t[:], in_=w12t_d[c * 128 : (c + 1) * 128, C4:KM])
        w2c.append(t)
    w3t = consts.tile([128, 128], BF16, tag="w3t")
    nc.scalar.dma_start(out=w3t[:], in_=w3t_d[:, :])
    w4t = []
    for q in range(2):
        t = consts.tile([128, C2], BF16, tag=f"w4t_{q}")
        nc.scalar.dma_start(out=t[:], in_=w4t_d[q * 128 : (q + 1) * 128, :])
        w4t.append(t)
    w5t = []
    for q in range(2):
        t = consts.tile([128, C], BF16, tag=f"w5t_{q}")
        nc.scalar.dma_start(out=t[:], in_=w5t_d[q * 128 : (q + 1) * 128, :])
        w5t.append(t)

    bias_t = {}
    if use_bias:
        b1_d, b3_d, b4_d, b5_d = aps["b1c"], aps["b3c"], aps["b4c"], aps["b5c"]
        b2row = consts.tile([1, C2], BF16, tag="b2row")
        nc.scalar.dma_start(out=b2row[:], in_=aps["b2row"][:, :])
        bias_t["b2row"] = b2row
        b1 = consts.tile([128, 1], F32, tag="b1")
        nc.scalar.dma_start(out=b1[:], in_=b1_d[:, :])
        bias_t["b1"] = b1
        b3 = consts.tile([128, 1], F32, tag="b3")
        nc.scalar.dma_start(out=b3[:], in_=b3_d[:, :])
        bias_t["b3"] = b3
        b4 = []
        for q in range(2):
            t = consts.tile([128, 1], F32, tag=f"b4_{q}")
            nc.scalar.dma_start(out=t[:], in_=b4_d[q * 128 : (q + 1) * 128, :])
            b4.append(t)
        bias_t["b4"] = b4
        b5 = []
        for oc in range(CK):
            t = consts.tile([128, 1], F32, tag=f"b5_{oc}")
            nc.scalar.dma_start(out=t[:], in_=b5_d[oc * 128 : (oc + 1) * 128, :])
            b5.append(t)
        bias_t["b5"] = b5

    sm = ctx.enter_context(tc.tile_pool(name="sm", bufs=1))

    # ---- phase 1: x1 k-major, PE transposes, A = x1 @ x^T, v = A @ w2^T ----
    with (
        tc.tile_pool(name="psA", bufs=2, space="PSUM") as psA,
        tc.tile_pool(name="psT", bufs=2, space="PSUM") as psT,
        tc.tile_pool(name="apsP", bufs=1, space="PSUM") as apsP,
        tc.tile_pool(name="vps", bufs=1, space="PSUM") as vps,
        tc.tile_pool(name="x18p", bufs=3) as x18p,
    ):
        A_ps = apsP.tile([128, C], F32, tag="A")
        v_ps = vps.tile([128, C2], F32, tag="v")

        # Warm the PE HAM clock-gate during the initial DMA window.
        wps = psA.tile([128, 128], F32, tag="warm", bufs=1)
        for _ in range(40):
            nc.tensor.matmul(wps[:], warm0[:], warm0[:], start=True, stop=True)

        def x1_group(t):
            px1 = psA.tile([128, 512], F32, tag="px1")
            for c in range(CK):
                nc.tensor.matmul(
                    px1[:],
                    w1c[c][:],
                    xs(c, t * 512, 512),
                    start=(c == 0),
                    stop=(c == CK - 1),
                )
            dstx1 = x1sb[:, t * 512 : (t + 1) * 512]
            if use_bias:
                nc.scalar.add(dstx1, px1[:], bias_t["b1"][:])
            elif t % 2 == 0:
                nc.scalar.copy(dstx1, px1[:])
            else:
                nc.vector.tensor_copy(dstx1, px1[:])

        def transp_group(t):
            pT = psT.tile([128, 512], BF16, tag="pT")
            for j in range(4):
                nc.tensor.transpose(
                    pT[:, j * 128 : (j + 1) * 128],
                    x1sb[:, t * 512 + j * 128 : t * 512 + (j + 1) * 128],
                    identb[:],
                )
            x18 = x18p.tile([128, 512], FP8, tag="x18")
            nc.vector.tensor_copy(x18[:], pT[:])
            return x18

        def a_group(t, x18):
            for j in range(4):
                s = t * 4 + j
                nc.tensor.matmul(
                    A_ps[:],
                    x18[:, j * 128 : (j + 1) * 128],
                    xt8[:, s * 512 : (s + 1) * 512],
                    start=(s == 0),
                    stop=(s == NSUB - 1),
                )

        # Interleaved warmups bridge the early DMA ramp (PE would starve
        # waiting for the first x blocks; idle >3.4us re-throttles HAM).
        PAD = {0: 16, 1: 14, 2: 12, 3: 10, 4: 8, 5: 6, 6: 4, 7: 2, 8: 1}
        pend = []
        for t in range(NT):
            x1_group(t)
            for _ in range(PAD.get(t, 0)):
                nc.tensor.matmul(
                    wps[:], warm0[:], warm0[:], start=True, stop=True
                )
            if t >= 1:
                pend.append((t - 1, transp_group(t - 1)))
            if t >= 2:
                a_group(*pend.pop(0))
        pend.append((NT - 1, transp_group(NT - 1)))
        while pend:
            a_group(*pend.pop(0))

        # ---- v = A @ w2^T (transpose A on the PE first) ----
        # Dummy matmuls fill each serial-wait gap so the PE never idles
        # long enough (>3.4us) to re-throttle the HAM clock.
        def warm_fill(n):
            for _ in range(n):
                nc.tensor.matmul(
                    wps[:], warm0[:], warm0[:], start=True, stop=True
                )

        asb = sm.tile([128, C], BF16, tag="asb")
        nc.scalar.copy(asb[:], A_ps[:])
        warm_fill(24)
        pTv = psT.tile([128, 512], BF16, tag="pT")
        for q in range(CK):
            nc.tensor.transpose(
                pTv[:, q * 128 : (q + 1) * 128],
                asb[:, q * 128 : (q + 1) * 128],
                identb[:],
            )
        warm_fill(24)
        atp = sm.tile([128, C], BF16, tag="atp")
        nc.vector.tensor_copy(atp[:], pTv[:])
        for q in range(CK):
            nc.tensor.matmul(
                v_ps[:],
                atp[:, q * 128 : (q + 1) * 128],
                w2c[q][:],
                start=(q == 0),
                stop=(q == CK - 1) and not use_bias,
            )
        if use_bias:
            # v = x1 @ (w2 x + b2)^T needs the rank-1 term rowsum(x1) (x) b2
            rs = sm.tile([128, 1], F32, tag="rs")
            nc.vector.tensor_reduce(rs[:], x1sb[:], axis=AX.X, op=AL.add)
            prs = psA.tile([128, 512], F32, tag="px1", name="prs")
            nc.tensor.transpose(prs[0:1, 0:128], rs[:], identf[:])
            rsT = sm.tile([1, 128], BF16, tag="rsT")
            nc.scalar.copy(rsT[:], prs[0:1, 0:128])
            nc.tensor.matmul(
                v_ps[:], rsT[:], bias_t["b2row"][:], start=False, stop=True
            )

        # ---- softmax over all 32768 entries of v ----
        m1 = sm.tile([128, 1], F32, tag="m1")
        nc.vector.tensor_reduce(m1[:], v_ps[:], axis=AX.X, op=AL.max)
        mall = sm.tile([128, 1], F32, tag="mall")
        nc.gpsimd.partition_all_reduce(mall[:], m1[:], 128, bass_isa.ReduceOp.max)
        negm = sm.tile([128, 1], F32, tag="negm")
        nc.vector.tensor_scalar_mul(negm[:], mall[:], -1.0)
        e = sm.tile([128, C2], BF16, tag="e")
        nc.scalar.activation(e[:], v_ps[:], AF.Exp, bias=negm[:], scale=1.0)

    s1 = sm.tile([128, 1], F32, tag="s1")
    nc.vector.tensor_reduce(s1[:], e[:], axis=AX.X, op=AL.add)
    sall = sm.tile([128, 1], F32, tag="sall")
    nc.gpsimd.partition_all_reduce(sall[:], s1[:], 128, bass_isa.ReduceOp.add)
    sinv = sm.tile([128, 1], F32, tag="sinv")
    nc.vector.reciprocal(sinv[:], sall[:])

    # ---- small chain: conv3+relu, n^T, z, W^T ----
    wt = sm.tile([128, C], BF16, tag="wt")
    with tc.tile_pool(name="psB", bufs=2, space="PSUM") as psB:
        # keep-warm dummies bridge the serial softmax window (shares the pT
        # tag's PSUM ring; the real transposes just rotate it later).
        # matmul is linear and sinv is a scalar, so w3 @ softmax(v) =
        # (w3 @ e) * sinv — the ps3 matmul needs only the exp output and
        # starts ~2us before sinv is ready.
        wps2 = psB.tile([128, 128], F32, tag="pT", name="warm2")
        for _ in range(28):
            nc.tensor.matmul(wps2[:], warm0[:], warm0[:], start=True, stop=True)

        ps3 = psB.tile([128, C2], F32, tag="ps3")
        nc.tensor.matmul(ps3[:], w3t[:], e[:], start=True, stop=True)
        warm3 = psB.tile([128, C2], F32, tag="ps3", name="warm3")
        for _ in range(40):
            nc.tensor.matmul(warm3[:, 0:128], warm0[:], warm0[:], start=True, stop=True)
        nsb = sm.tile([128, C2], F32, tag="nsb")
        # nsb = relu((ps3 + e) * sinv (+ b3))
        nc.vector.tensor_tensor(nsb[:], ps3[:], e[:], op=AL.add)
        nc.vector.tensor_scalar_mul(nsb[:], nsb[:], sinv[:])
        if use_bias:
            nc.vector.tensor_scalar_add(nsb[:], nsb[:], bias_t["b3"][:])
        nc.vector.tensor_scalar_max(nsb[:], nsb[:], 0.0)

        nts = []
        for q in range(2):
            pT = psB.tile([128, 128], F32, tag="pT")
            nc.tensor.transpose(pT[:], nsb[:, q * 128 : (q + 1) * 128], identf[:])
            ntq = sm.tile([128, 128], BF16, tag=f"nt{q}")
            nc.scalar.copy(ntq[:], pT[:])
            nts.append(ntq)
            for _ in range(12):
                nc.tensor.matmul(
                    warm3[:, 0:128], warm0[:], warm0[:], start=True, stop=True
                )

        zs = []
        for mc in range(2):
            pz = psB.tile([128, 128], F32, tag="pz")
            for q in range(2):
                nc.tensor.matmul(
                    pz[:],
                    w4t[q][:, mc * 128 : (mc + 1) * 128],
                    nts[q][:],
                    start=(q == 0),
                    stop=(q == 1),
                )
            zq = sm.tile([128, 128], BF16, tag=f"z{mc}")
            if use_bias:
                nc.scalar.add(zq[:], pz[:], bias_t["b4"][mc][:])
            else:
                nc.scalar.copy(zq[:], pz[:])
            zs.append(zq)
            for _ in range(4):
                nc.tensor.matmul(
                    warm3[:, 0:128], warm0[:], warm0[:], start=True, stop=True
                )

        pW = psB.tile([128, C], F32, tag="pW")
        for mc in range(2):
            nc.tensor.matmul(
                pW[:], zs[mc][:], w5t[mc][:], start=(mc == 0), stop=(mc == 1)
            )
        nc.scalar.copy(wt[:], pW[:])

    # ---- phase 2: x_res = W @ x1 (x1 already resident), residual, out ----
    # Output staged in [128, 1536] bf16 tiles; all 24 out-DMAs on sync.
    # Residual alternates PE (identity matmul into the x_res PSUM bank +
    # ACT copy to staging) and DVE (tensor_tensor writes staging directly).
    with (
        tc.tile_pool(name="psD", bufs=6, space="PSUM") as psD,
        tc.tile_pool(name="outp", bufs=2) as outp,
    ):
        stage = {}

        for t in range(NT):
            g, gi = divmod(t, 3)
            x1t = x1sb[:, t * 512 : (t + 1) * 512]
            for oc in range(CK):
                if gi == 0:
                    stage[oc] = outp.tile(
                        [128, 1536], BF16, tag=f"st{oc}", name=f"st{oc}_{g}"
                    )
                st = stage[oc]
                dst = st[:, gi * 512 : (gi + 1) * 512]
                pr = psD.tile([128, 512], F32, tag="pr")
                use_pe = (t * CK + oc) % 2 == 0
                nc.tensor.matmul(
                    pr[:],
                    wt[:, oc * 128 : (oc + 1) * 128],
                    x1t,
                    start=True,
                    stop=not use_pe,
                )
                b5s = bias_t["b5"][oc][:] if use_bias else None
                if use_pe:
                    nc.tensor.matmul(
                        pr[:],
                        identb[:],
                        xs(oc, t * 512, 512),
                        start=False,
                        stop=True,
                    )
                    if b5s is not None:
                        nc.scalar.add(dst, pr[:], b5s)
                    else:
                        nc.scalar.copy(dst, pr[:])
                else:
                    if b5s is not None:
                        nc.vector.scalar_tensor_tensor(
                            dst, pr[:], b5s, xs(oc, t * 512, 512),
                            op0=AL.add, op1=AL.add,
                        )
                    else:
                        nc.vector.tensor_tensor(
                            dst, pr[:], xs(oc, t * 512, 512), op=AL.add
                        )
                if gi == 2:
                    nc.sync.dma_start(
                        out=out_d[
                            oc * 128 : (oc + 1) * 128, g * 1536 : (g + 1) * 1536
                        ],
                        in_=st[:],
                    )


def _build(use_bias):
    nc = bacc.Bacc("TRN2", target_bir_lowering=False, debug=False, num_devices=N_CORES)
    aps = {
        "x": nc.dram_tensor("x", [128, XCOLS], BF16, kind="ExternalInput").ap(),
        "xt8": nc.dram_tensor("xt8", [128, XCOLS], FP8, kind="ExternalInput").ap(),
        "w12t": nc.dram_tensor("w12t", [C, KM], BF16, kind="ExternalInput").ap(),
        "w3t": nc.dram_tensor("w3t", [C4, C4], BF16, kind="ExternalInput").ap(),
        "w4t": nc.dram_tensor("w4t", [C2, C2], BF16, kind="ExternalInput").ap(),
        "w5t": nc.dram_tensor("w5t", [C2, C], BF16, kind="ExternalInput").ap(),
        "identf": nc.dram_tensor("identf", [128, 128], F32, kind="ExternalInput").ap(),
        "identb": nc.dram_tensor("identb", [128, 128], BF16, kind="ExternalInput").ap(),
        "out": nc.dram_tensor("out", [C, HW], BF16, kind="ExternalOutput").ap(),
    }
    if use_bias:
        aps["b2row"] = nc.dram_tensor(
            "b2row", [1, C2], BF16, kind="ExternalInput"
        ).ap()
        aps["b1c"] = nc.dram_tensor("b1c", [C4, 1], F32, kind="ExternalInput").ap()
        aps["b3c"] = nc.dram_tensor("b3c", [C4, 1], F32, kind="ExternalInput").ap()
        aps["b4c"] = nc.dram_tensor("b4c", [C2, 1], F32, kind="ExternalInput").ap()
        aps["b5c"] = nc.dram_tensor("b5c", [C, 1], F32, kind="ExternalInput").ap()

    from contextlib import ExitStack

    with tile.TileContext(nc) as tc:
        with ExitStack() as ctx:
            _emit(ctx, tc, aps, use_bias)
    nc.compile()
    return nc


_CACHE = {}


def _pack_x(xb_bf):
    """[512, 9216] bf16 -> packed [128, 36864]: block0 as 3 sub-blocks of
    (c x 512), blocks 1..5 as (c x 1536)."""
    xc = xb_bf.reshape(CK, 128, HW)
    parts = []
    for sub in range(3):
        parts.append(xc[:, :, sub * 512 : (sub + 1) * 512])
    for b in range(1, NBLK):
        parts.append(xc[:, :, b * BLK : (b + 1) * BLK])
    return np.concatenate(
        [p.transpose(1, 0, 2).reshape(128, -1) for p in parts], axis=1
    )


def _pack_xt8(xb):
    """[512, 9216] f32 -> fp8 x^T packed [128, 36864]:
    col s*512 + cc holds x[cc, s*128 + p] for partition p."""
    xt = xb.reshape(C, NSUB, 128).transpose(2, 1, 0)  # [128, 72, 512]
    return np.ascontiguousarray(
        xt.reshape(128, XCOLS).astype(ml_dtypes.float8_e4m3)
    )


def _run(inputs, trace=False, **run_kwargs):
    x = np.ascontiguousarray(np.asarray(inputs["x"], dtype=np.float32))
    assert x.shape == (N_CORES, C, H, W_IMG), x.shape
    w1 = np.asarray(inputs["w1"], dtype=np.float32)
    w2 = np.asarray(inputs["w2"], dtype=np.float32)
    w3 = np.asarray(inputs["w3"], dtype=np.float32)
    w4 = np.asarray(inputs["w4"], dtype=np.float32)
    w5 = np.asarray(inputs["w5"], dtype=np.float32)
    b1 = np.asarray(inputs["b1"], dtype=np.float32)
    b2 = np.asarray(inputs["b2"], dtype=np.float32)
    b3 = np.asarray(inputs["b3"], dtype=np.float32)
    b4 = np.asarray(inputs["b4"], dtype=np.float32)
    b5 = np.asarray(inputs["b5"], dtype=np.float32)
    use_bias = bool(
        np.any(b1) or np.any(b2) or np.any(b3) or np.any(b4) or np.any(b5)
    )

    if use_bias not in _CACHE:
        _CACHE[use_bias] = _build(use_bias)
    nc = _CACHE[use_bias]

    bf = ml_dtypes.bfloat16
    w12t = np.ascontiguousarray(
        np.concatenate([w1.T, w2.T], axis=1), dtype=bf
    )  # [512, 384]
    w3t = np.ascontiguousarray(w3.T, dtype=bf)
    w4t = np.ascontiguousarray(w4.T, dtype=bf)
    w5t = np.ascontiguousarray(w5.T, dtype=bf)

    shared = {
        "w12t": w12t,
        "w3t": w3t,
        "w4t": w4t,
        "w5t": w5t,
        "identf": np.eye(128, dtype=np.float32),
        "identb": np.eye(128, dtype=bf),
    }
    if use_bias:
        shared["b2row"] = np.ascontiguousarray(b2[None, :], dtype=bf)
        shared["b1c"] = np.ascontiguousarray(b1[:, None])
        shared["b3c"] = np.ascontiguousarray(b3[:, None])
        shared["b4c"] = np.ascontiguousarray(b4[:, None])
        shared["b5c"] = np.ascontiguousarray(b5[:, None])

    in_maps = []
    for b in range(N_CORES):
        xb = x[b].reshape(C, HW)
        in_maps.append(
            {
                "x": np.ascontiguousarray(_pack_x(xb.astype(bf))),
                "xt8": _pack_xt8(xb.astype(bf).astype(np.float32)),
                **shared,
            }
        )
    res = run_bass_kernel_spmd(
        nc, in_maps, core_ids=list(range(N_CORES)), trace=trace, **run_kwargs
    )
    out = np.stack(
        [
            np.asarray(res.results[b]["out"]).astype(np.float32).reshape(C, H, W_IMG)
            for b in range(N_CORES)
        ]
    )
    return out, res


def kernel(**inputs):
    out, _ = _run(inputs, trace=False)
    return out
